# revision 63
# baseline (speedup 1.0000x reference)
"""Trainium2 Bass kernel for a Swin-style local-window ViT block.

Problem (hardcoded): x (4, 256, 256, 96) fp32, 8x8 windows, 3 heads (hd=32),
LN -> window attention (+rel-pos bias) -> proj -> residual -> LN -> MLP(4x,
gelu) -> residual.

Sharding: data-parallel. (B*H)=1024 image rows are split into 8 slabs of 128
rows; each slab holds 512 complete 8x8 windows, so the 8 cores are fully
independent (weights replicated).

Host-side, x is pre-permuted into windowed token order
[band, token-partition, group-in-band, window-pair, ch] so every DMA is a
large contiguous transfer (6 KB per partition per band); the output is
stored in the same layout and inverse-permuted on the host. The attention
residual stream x2 lives entirely in SBUF (no DRAM round trip).

Per-core program (built once, run SPMD on 8 cores):
  Phase A (attention), per 512-token group (8 windows, 4 window-pairs):
    band DMA (1 per 4 groups) -> batched bn_stats/per-window bn_aggr -> rstd
    batched per 8 groups via Ln+Exp (stays on the exp ACT table) -> LN apply
    (bf16) -> PE transpose hT -> qkv matmuls -> per-(window,head) score
    matmuls (scores^T so softmax normalization lands token-major) -> one ACT
    Exp -> multiply by host precomputed exp(bias^T) table -> attn@v and
    row-sum matmuls (shared stationary) -> reciprocal + normalize -> PE
    transpose -> proj -> residual -> x2_all in SBUF.
  Phase B (MLP): LN2 (stats from phase A, rstd batched once so the ACT table
    switches exactly once to gelu) -> fc1 (transposed form, N=512) -> one ACT
    Gelu -> fc2 -> residual -> band store DMA (1 per 4 groups).

LayerNorm gamma/beta and the attention scale are folded into the weights on
the host; all bias vectors in this problem are zero (asserted)."""

import sys

sys.path.insert(0, "/opt/trn_rl_repo")

import numpy as np

import concourse.bass as bass
import concourse.bacc as bacc
import concourse.tile as tile
from concourse import mybir
from concourse import bass_utils

F32 = mybir.dt.float32
BF16 = mybir.dt.bfloat16
AF = mybir.ActivationFunctionType
ALU = mybir.AluOpType

B, H, W, C = 4, 256, 256, 96
WIN = 8
HEADS = 3
HD = 32
SCALE = HD ** -0.5
HID = 4 * C

NCORES = 8
ROWS = (B * H) // NCORES          # 128 image rows per core
NGROUPS = 64                      # groups of 512 tokens (8 windows) per core
SUPER = 8                         # groups per rstd batch
EPS = 1e-5

NBANDS = 16
BANDG = 4                         # groups per band
BAND_FREE = BANDG * 4 * C         # 1536 free els per partition per band


def _rel_pos_index():
    coords = np.stack(np.meshgrid(np.arange(WIN), np.arange(WIN), indexing="ij")).reshape(2, -1)
    rel = coords[:, :, None] - coords[:, None, :]
    rel = rel.transpose(1, 2, 0).astype(np.int64)
    rel[:, :, 0] += WIN - 1
    rel[:, :, 1] += WIN - 1
    rel[:, :, 0] *= 2 * WIN - 1
    return rel.sum(-1)  # (64, 64)


REL_IDX = _rel_pos_index()

_CACHE = {}


def _band_dram_ap(handle, band):
    return bass.AP(tensor=handle, offset=band * 128 * BAND_FREE,
                   ap=[[BAND_FREE, 128], [1, BAND_FREE]])


def _build_program(ngroups=NGROUPS, super_=SUPER, act_fn=AF.Gelu, phases="AB", astage=99, use_tp=True):
    nc = bacc.Bacc("TRN2", target_bir_lowering=False, debug=False)

    # Host pre-permuted windowed layout: [band, partition(token), group, wp, ch]
    x_h = nc.dram_tensor("x", [NBANDS, 128, BANDG, 4, C], F32, kind="ExternalInput")
    out_h = nc.dram_tensor("out", [NBANDS, 128, BANDG, 4, C], F32, kind="ExternalOutput")

    wqkv_h = nc.dram_tensor("wqkv", [C, 3 * C], BF16, kind="ExternalInput")
    wp_h = nc.dram_tensor("wproj", [C, C], BF16, kind="ExternalInput")
    w1_h = nc.dram_tensor("w1", [C, HID], BF16, kind="ExternalInput")
    w2_h = nc.dram_tensor("w2", [3, 128, C], BF16, kind="ExternalInput")
    biasimg_h = nc.dram_tensor("biasimg", [128, 3, 512], BF16, kind="ExternalInput")
    ident_h = nc.dram_tensor("ident", [128, 128], BF16, kind="ExternalInput")

    with tile.TileContext(nc) as tc:
        with tc.tile_pool(name="const", bufs=1) as cpool:
            wqkv = cpool.tile([C, 3 * C], BF16)
            nc.sync.dma_start(out=wqkv, in_=wqkv_h.ap())
            wproj = cpool.tile([C, C], BF16)
            nc.sync.dma_start(out=wproj, in_=wp_h.ap())
            w1 = cpool.tile([C, HID], BF16)
            nc.sync.dma_start(out=w1, in_=w1_h.ap())
            w2 = cpool.tile([128, 3, C], BF16)
            nc.sync.dma_start(out=w2, in_=w2_h.ap().rearrange("c p f -> p c f"))
            biasimg = cpool.tile([128, 3, 512], BF16)
            nc.sync.dma_start(out=biasimg, in_=biasimg_h.ap())
            ident = cpool.tile([128, 128], BF16)
            nc.sync.dma_start(out=ident, in_=ident_h.ap())
            ones32 = cpool.tile([128, HD], BF16)
            nc.vector.memset(ones32, 1.0)
            epsb = cpool.tile([128, 1], F32)
            nc.vector.memset(epsb, EPS)
            ln05b = cpool.tile([128, 1], F32)
            nc.vector.memset(ln05b, -0.6931471805599453)
            # attention residual stream, SBUF-resident for the whole kernel
            x2_all = cpool.tile([128, ngroups, 4, C], BF16)
            if astage >= 8:
                mv2_all = cpool.tile([128, ngroups, 4, 2], F32)
                rstd2_all = cpool.tile([128, ngroups * 4], F32)

            # Warm-up: make PE/DVE observe each const-load DMA semaphore via a
            # tiny op, so real instructions never need two sync waits (this
            # walrus build supports at most one per instruction).
            with tc.tile_pool(name="warm", bufs=1, space="PSUM") as wps:
                wp_t = wps.tile([1, 8], F32)
                def _tiny(t):
                    base = t[:]
                    return bass.AP(tensor=base.tensor, offset=base.offset,
                                   ap=[[base.ap[0][0], 1], [1, 1]])
                for ci, cst in enumerate((wqkv, wproj, w1, w2, ident)):
                    nc.tensor.matmul(wp_t[0:1, ci:ci + 1], _tiny(cst),
                                     _tiny(cst), start=True, stop=True)
                wd = cpool.tile([1, 1], BF16)
                nc.vector.tensor_copy(out=wd, in_=_tiny(biasimg))

            # ---------------- Phase A: attention ----------------
            with (
                tc.tile_pool(name="xin", bufs=4) as xpool,
                tc.tile_pool(name="stat", bufs=6) as stpool,
                tc.tile_pool(name="mv1", bufs=3) as mvpool,
                tc.tile_pool(name="sba", bufs=3) as sba,
                tc.tile_pool(name="psT", bufs=2, space="PSUM") as psT,
                tc.tile_pool(name="psM", bufs=3, space="PSUM") as psM,
                tc.tile_pool(name="psS", bufs=1, space="PSUM") as psS,
            ):
                band_tiles = {}
                for sb in range(ngroups // super_):
                    x_ts = []
                    mv1 = mvpool.tile([128, super_, 4, 2], F32, tag="mv1")
                    for gi in range(super_):
                        g = sb * super_ + gi
                        if g % BANDG == 0:
                            band = g // BANDG
                            xb = xpool.tile([128, BANDG, 4, C], F32, tag="xband")
                            nc.sync.dma_start(out=xb, in_=_band_dram_ap(x_h, band))
                            band_tiles[band] = xb
                        x_t = band_tiles[g // BANDG][:, g % BANDG, :, :]
                        st = stpool.tile([128, 4, 6], F32, tag="st")
                        for j in range(4):
                            nc.vector.bn_stats(out=st[:, j, :], in_=x_t[:, j, :])
                        # even/odd halves have equal counts (48): mean-sum and
                        # (count*var)-sum replace bn_aggr; the /2 and /96 are
                        # folded into the batched ACT rstd and the apply STT.
                        stb = st[:]
                        def _stp(off):
                            return bass.AP(tensor=stb.tensor, offset=stb.offset + off,
                                           ap=[stb.ap[0], [6, 4]])
                        nc.vector.tensor_tensor(out=mv1[:, gi, :, 0], in0=_stp(1),
                                                in1=_stp(4), op=ALU.add)
                        nc.vector.tensor_tensor(out=mv1[:, gi, :, 1], in0=_stp(2),
                                                in1=_stp(5), op=ALU.add)
                        x_ts.append(x_t)
                    # batched rstd for SUPER groups: rstd = exp(-0.5*ln(var+eps))
                    var_ap = bass.AP(
                        tensor=mv1.tensor,
                        offset=mv1[:].offset + 1,
                        ap=[mv1[:].ap[0], [8, super_], [2, 4], [1, 1]],
                    )
                    lnv = stpool.tile([128, super_ * 4], F32, tag="lnv")
                    nc.scalar.activation(out=lnv, in_=var_ap, func=AF.Ln, bias=epsb[:],
                                         scale=1.0 / 96.0)
                    rstd1 = stpool.tile([128, super_ * 4], F32, tag="rstd1")
                    nc.scalar.activation(out=rstd1, in_=lnv[:], func=AF.Exp, scale=-0.5,
                                         bias=ln05b[:])

                    for gi in range(super_):
                        g = sb * super_ + gi
                        x_t = x_ts[gi]
                        if astage < 1:
                            continue
                        # LN1 apply: two batched TTs with stride-0 free-dim
                        # broadcast of the per-(token, window) stats.
                        mean_b = bass.AP(
                            tensor=mv1.tensor,
                            offset=mv1[:].offset + gi * 8,
                            ap=[mv1[:].ap[0], [2, 4], [0, C]],
                        )
                        rstd_b = bass.AP(
                            tensor=rstd1.tensor,
                            offset=rstd1[:].offset + gi * 4,
                            ap=[rstd1[:].ap[0], [1, 4], [0, C]],
                        )
                        xc = sba.tile([128, 4, C], F32, tag="xc")
                        nc.vector.scalar_tensor_tensor(out=xc, in0=x_t, scalar=2.0,
                                                       in1=mean_b, op0=ALU.mult,
                                                       op1=ALU.subtract)
                        h_t = sba.tile([128, 4, C], BF16, tag="h")
                        nc.vector.tensor_tensor(out=h_t, in0=xc[:], in1=rstd_b, op=ALU.mult)
                        if astage < 2:
                            continue
                        hT_ps = psT.tile([C, 512], BF16, tag="tp")
                        for j in range(4):
                            nc.tensor.transpose(hT_ps[:, j * 128:(j + 1) * 128], h_t[:, j, :], ident[:])
                        hT = sba.tile([C, 512], BF16, tag="hT")
                        nc.vector.tensor_copy(out=hT, in_=hT_ps[:])

                        if astage < 3:
                            continue
                        qT_ps = psM.tile([C, 512], F32, tag="m")
                        nc.tensor.matmul(qT_ps[:], wqkv[:, 0:C], hT[:], start=True, stop=True)
                        kT_ps = psM.tile([C, 512], F32, tag="m")
                        nc.tensor.matmul(kT_ps[:], wqkv[:, C:2 * C], hT[:], start=True, stop=True)
                        v_ps = psM.tile([128, 4, C], F32, tag="m")
                        for j in range(4):
                            nc.tensor.matmul(v_ps[:, j, :], hT[:, j * 128:(j + 1) * 128],
                                             wqkv[:, 2 * C:3 * C], start=True, stop=True)
                        qT = sba.tile([C, 512], BF16, tag="qT")
                        nc.scalar.activation(out=qT, in_=qT_ps[:], func=AF.Copy, bias=0.0)
                        kT = sba.tile([C, 512], BF16, tag="kT")
                        nc.scalar.activation(out=kT, in_=kT_ps[:], func=AF.Copy, bias=0.0)
                        v_t = sba.tile([128, 4, C], BF16, tag="v")
                        nc.vector.tensor_copy(out=v_t, in_=v_ps[:])

                        if astage < 4:
                            continue
                        # Diagonal score layout: window a's keys on partitions
                        # 0:64, window b's on 64:128; the shared free column
                        # (wp, q) indexes "query q of window a" on the top half
                        # and "of window b" on the bottom half. Bias is
                        # preloaded into PSUM via PE (sets has_written), scores
                        # accumulate on top, so exp(s+b) needs no extra multiply.
                        # bias preload: copy the rel-pos bias image (with -30 on
                        # cross-window blocks) into PSUM via PE so has_written is
                        # set, then accumulate scores on top; exp(s+b) replaces
                        # an extra DVE multiply.
                        sc_ps = psS.tile([128, 3, 512], F32, tag="sc")

                        def _flat(t, lo, n):
                            base = t[:]
                            return bass.AP(tensor=base.tensor, offset=base.offset + lo,
                                           ap=[base.ap[0], [1, n]])
                        for bk in range(3):
                            nc.tensor.matmul(_flat(sc_ps, bk * 512, 512), ident[:],
                                             _flat(biasimg, bk * 512, 512),
                                             start=True, stop=False)
                        for wp in range(4):
                            for hh in range(HEADS):
                                t0 = wp * 128
                                nc.tensor.matmul(
                                    sc_ps[:, hh, t0:t0 + 128],
                                    kT[hh * HD:(hh + 1) * HD, t0:t0 + 128],
                                    qT[hh * HD:(hh + 1) * HD, t0:t0 + 128],
                                    start=False, stop=(wp == 3),
                                    tile_position=(hh * HD, 0) if use_tp else None,
                                )
                        if astage < 5:
                            continue
                        E_t = sba.tile([128, 3, 512], BF16, tag="E")
                        nc.scalar.activation(out=E_t[:, :, 0:256], in_=sc_ps[:, :, 0:256],
                                             func=AF.Exp)
                        nc.scalar.activation(out=E_t[:, :, 256:512], in_=sc_ps[:, :, 256:512],
                                             func=AF.Exp)

                        if astage < 6:
                            continue
                        o_ps = psM.tile([128, 4, C], F32, tag="m")
                        s_ps = psM.tile([128, 12], F32, tag="m")
                        for wp in range(4):
                            for hh in range(HEADS):
                                for par in range(2):
                                    p0 = par * 64
                                    t0 = wp * 128 + par * 64
                                    nc.tensor.matmul(
                                        o_ps[p0:p0 + 64, wp, hh * HD:(hh + 1) * HD],
                                        E_t[:, hh, t0:t0 + 64],
                                        v_t[:, wp, hh * HD:(hh + 1) * HD],
                                        start=True, stop=True,
                                        tile_position=(0, p0) if use_tp else None,
                                    )
                                    nc.tensor.matmul(
                                        s_ps[p0:p0 + 64, wp * 3 + hh:wp * 3 + hh + 1],
                                        E_t[:, hh, t0:t0 + 64],
                                        ones32[:, 0:1],
                                        start=True, stop=True,
                                        tile_position=(0, p0) if use_tp else None,
                                    )
                        rs = stpool.tile([128, 12], F32, tag="rs")
                        nc.vector.reciprocal(out=rs, in_=s_ps[:])
                        o_t = sba.tile([128, 4, C], BF16, tag="o")
                        rs_b = bass.AP(
                            tensor=rs.tensor, offset=rs[:].offset,
                            ap=[rs[:].ap[0], [3, 4], [1, 3], [0, HD]],
                        )
                        o_src = o_ps[:].rearrange("p a (h d) -> p a h d", h=HEADS)
                        o_dst = o_t[:].rearrange("p a (h d) -> p a h d", h=HEADS)
                        nc.vector.tensor_tensor(out=o_dst, in0=o_src, in1=rs_b, op=ALU.mult)

                        oT_ps = psT.tile([C, 512], BF16, tag="tp")
                        for j in range(4):
                            nc.tensor.transpose(oT_ps[:, j * 128:(j + 1) * 128], o_t[:, j, :], ident[:])
                        oT = sba.tile([C, 512], BF16, tag="oT")
                        nc.vector.tensor_copy(out=oT, in_=oT_ps[:])

                        if astage < 7:
                            continue
                        att_ps = psM.tile([128, 4, C], F32, tag="m")
                        for j in range(4):
                            nc.tensor.matmul(att_ps[:, j, :], oT[:, j * 128:(j + 1) * 128],
                                             wproj[:], start=True, stop=True)
                        if astage < 8:
                            continue
                        nc.vector.scalar_tensor_tensor(
                            out=x2_all[:, g, :, :], in0=att_ps[:], scalar=1.0, in1=x_t,
                            op0=ALU.mult, op1=ALU.add,
                        )
                        st2 = stpool.tile([128, 4, 6], F32, tag="st")
                        for j in range(4):
                            nc.vector.bn_stats(out=st2[:, j, :], in_=x2_all[:, g, j, :])
                        st2b = st2[:]
                        def _st2p(off):
                            return bass.AP(tensor=st2b.tensor, offset=st2b.offset + off,
                                           ap=[st2b.ap[0], [6, 4]])
                        nc.vector.tensor_tensor(out=mv2_all[:, g, :, 0], in0=_st2p(1),
                                                in1=_st2p(4), op=ALU.add)
                        nc.vector.tensor_tensor(out=mv2_all[:, g, :, 1], in0=_st2p(2),
                                                in1=_st2p(5), op=ALU.add)

                # batched LN2 rstd (still on the exp/ln ACT table)
                if astage >= 8:
                    var2_ap = bass.AP(
                        tensor=mv2_all.tensor,
                        offset=mv2_all[:].offset + 1,
                        ap=[mv2_all[:].ap[0], [8, ngroups], [2, 4], [1, 1]],
                    )
                    lnv2 = cpool.tile([128, ngroups * 4], F32)
                    nc.scalar.activation(out=lnv2, in_=var2_ap, func=AF.Ln, bias=epsb[:],
                                         scale=1.0 / 96.0)
                    nc.scalar.activation(out=rstd2_all[:], in_=lnv2[:], func=AF.Exp,
                                         scale=-0.5, bias=ln05b[:])

            # ---------------- Phase B: MLP ----------------
            do_b = "B" in phases and astage >= 8
            with (
                tc.tile_pool(name="oband", bufs=3) as opool,
                tc.tile_pool(name="sbb", bufs=3) as sbb,
                tc.tile_pool(name="psT2", bufs=1, space="PSUM") as psT2,
                tc.tile_pool(name="psG", bufs=2, space="PSUM") as psG,
                tc.tile_pool(name="psF", bufs=1, space="PSUM") as psF,
            ):
                ob = None
                for g in range(ngroups if do_b else 0):
                    if g % BANDG == 0:
                        ob = opool.tile([128, BANDG, 4, C], F32, tag="oband")
                    x2_t = x2_all[:, g, :, :]
                    mean2_b = bass.AP(
                        tensor=mv2_all.tensor,
                        offset=mv2_all[:].offset + g * 8,
                        ap=[mv2_all[:].ap[0], [2, 4], [0, C]],
                    )
                    rstd2_b = bass.AP(
                        tensor=rstd2_all.tensor,
                        offset=rstd2_all[:].offset + g * 4,
                        ap=[rstd2_all[:].ap[0], [1, 4], [0, C]],
                    )
                    x2c = sbb.tile([128, 4, C], F32, tag="x2c")
                    nc.vector.scalar_tensor_tensor(out=x2c, in0=x2_t, scalar=2.0,
                                                   in1=mean2_b, op0=ALU.mult,
                                                   op1=ALU.subtract)
                    h2 = sbb.tile([128, 4, C], BF16, tag="h2")
                    nc.vector.tensor_tensor(out=h2, in0=x2c[:], in1=rstd2_b, op=ALU.mult)
                    h2T_ps = psT2.tile([C, 512], BF16, tag="tp2")
                    for j in range(4):
                        nc.tensor.transpose(h2T_ps[:, j * 128:(j + 1) * 128], h2[:, j, :], ident[:])
                    h2T = sbb.tile([C, 512], BF16, tag="h2T")
                    nc.vector.tensor_copy(out=h2T, in_=h2T_ps[:])

                    g1_ps = psG.tile([128, 3, 512], F32, tag="g1")
                    for ch in range(3):
                        nc.tensor.matmul(g1_ps[:, ch, :], w1[:, ch * 128:(ch + 1) * 128],
                                         h2T[:], start=True, stop=True)
                    g1 = sbb.tile([128, 3, 512], BF16, tag="g1s")
                    nc.scalar.activation(out=g1, in_=g1_ps[:], func=act_fn)

                    f2_ps = psF.tile([128, 4, C], F32, tag="f2")
                    for j in range(4):
                        for ch in range(3):
                            nc.tensor.matmul(
                                f2_ps[:, j, :],
                                g1[:, ch, j * 128:(j + 1) * 128],
                                w2[:, ch, :],
                                start=(ch == 0), stop=(ch == 2),
                            )
                    nc.vector.scalar_tensor_tensor(
                        out=ob[:, g % BANDG, :, :], in0=f2_ps[:], scalar=1.0, in1=x2_t,
                        op0=ALU.mult, op1=ALU.add,
                    )
                    if g % BANDG == BANDG - 1:
                        nc.sync.dma_start(out=_band_dram_ap(out_h, g // BANDG), in_=ob[:])

    nc.compile()
    return nc


def _get_program():
    if "nc" not in _CACHE:
        _CACHE["nc"] = _build_program()
    return _CACHE["nc"]


def _prep_consts(norm1_g, norm1_b, qkv_w, qkv_b, proj_w, proj_b,
                 rel_bias_table, norm2_g, norm2_b, fc1_w, fc1_b, fc2_w, fc2_b):
    # Fold LN1 affine into qkv weights; fold attention scale into the q part.
    wqkv = qkv_w * norm1_g[:, None]
    bqkv = norm1_b @ qkv_w + qkv_b            # (288,)
    wqkv = wqkv.copy()
    wqkv[:, 0:C] *= SCALE
    bqkv = bqkv.copy()
    bqkv[0:C] *= SCALE
    # Column order of qkv_w is [(q|k|v) major, head, hd] per the reference
    # reshape (Bw, N, 3, HEADS, HD): q = cols 0:96, k = 96:192, v = 192:288.
    assert np.allclose(bqkv[2 * C:], 0.0), "nonzero v bias not supported"
    assert np.allclose(proj_b, 0.0) and np.allclose(fc1_b, 0.0) and np.allclose(fc2_b, 0.0), \
        "nonzero proj/fc biases not supported"
    assert np.allclose(bqkv[0:2 * C], 0.0), "nonzero q/k bias not supported"

    w1 = fc1_w * norm2_g[:, None]
    b1 = norm2_b @ fc1_w + fc1_b
    assert np.allclose(b1, 0.0), "nonzero folded fc1 bias not supported"

    # transposed rel-pos bias image, replicated for both windows of a pair and
    # for all 4 pairs; cross-window key/query blocks get -30 so exp() of the
    # preloaded-then-accumulated scores vanishes there.
    bias = rel_bias_table[REL_IDX]            # (64, 64, HEADS) [q, k, h]
    biasT = bias.transpose(2, 1, 0)           # (h, k, q)
    blk = np.full((128, 3, 512), -30.0, np.float32)
    for wp in range(4):
        for hh in range(HEADS):
            for par in range(2):
                p0 = par * 64
                t0 = wp * 128 + par * 64
                blk[p0:p0 + 64, hh, t0:t0 + 64] = biasT[hh]

    w2 = fc2_w.reshape(3, 128, C)

    return {
        "wqkv": wqkv.astype(np.float32),
        "wproj": proj_w,
        "w1": w1,
        "w2": w2,
        "biasimg": blk,
    }


def _to_bf16(a):
    import ml_dtypes
    return np.asarray(a, dtype=np.float32).astype(ml_dtypes.bfloat16)



def _permute_x(slab):
    """[128, 256, 96] raster -> [16 band, 128 token, 4 group, 4 wp, 96]."""
    xp = slab.reshape(NBANDS, 8, 4, 4, 2, 8, C).transpose(0, 4, 1, 5, 2, 3, 6)
    return np.ascontiguousarray(xp).reshape(NBANDS, 128, BANDG, 4, C)


def _unpermute_out(o):
    """[16, 128, 4, 4, 96] windowed -> [128, 256, 96] raster."""
    o = o.reshape(NBANDS, 2, 8, 8, 4, 4, C).transpose(0, 2, 4, 5, 1, 3, 6)
    return o.reshape(128, 256, C)


TRACE = False
LAST_RESULT = {}


def kernel(**inputs):
    x = np.asarray(inputs["x"], np.float32)
    consts = _prep_consts(
        np.asarray(inputs["norm1_g"], np.float32), np.asarray(inputs["norm1_b"], np.float32),
        np.asarray(inputs["qkv_w"], np.float32), np.asarray(inputs["qkv_b"], np.float32),
        np.asarray(inputs["proj_w"], np.float32), np.asarray(inputs["proj_b"], np.float32),
        np.asarray(inputs["rel_bias_table"], np.float32),
        np.asarray(inputs["norm2_g"], np.float32), np.asarray(inputs["norm2_b"], np.float32),
        np.asarray(inputs["fc1_w"], np.float32), np.asarray(inputs["fc1_b"], np.float32),
        np.asarray(inputs["fc2_w"], np.float32), np.asarray(inputs["fc2_b"], np.float32),
    )

    shared = {
        "wqkv": _to_bf16(consts["wqkv"]),
        "wproj": _to_bf16(consts["wproj"]),
        "w1": _to_bf16(consts["w1"]),
        "w2": _to_bf16(consts["w2"]),
        "biasimg": _to_bf16(consts["biasimg"]),
        "ident": _to_bf16(np.eye(128, dtype=np.float32)),
    }

    xr = x.reshape(B * H, W, C)
    in_maps = []
    for c in range(NCORES):
        m = dict(shared)
        m["x"] = _permute_x(xr[c * ROWS:(c + 1) * ROWS])
        in_maps.append(m)

    nc = _get_program()
    res = bass_utils.run_bass_kernel_spmd(
        nc, in_maps, core_ids=list(range(NCORES)), trace=TRACE)
    if TRACE:
        LAST_RESULT["exec_time_ns"] = res.exec_time_ns
        LAST_RESULT["profile_json"] = res.profile_json
        LAST_RESULT["trace"] = res.instructions_and_trace
    out = np.concatenate([_unpermute_out(r["out"]) for r in res.results], axis=0)
    return out.reshape(B, H, W, C)


if __name__ == "__main__":
    rng = np.random.default_rng(0)
    print("building program...")
    _get_program()
    print("program built ok")
